# revision 9
# baseline (speedup 1.0000x reference)
"""Trainium2 Bass kernel for ContextAttentionMaskLuong (v3: fp16 streaming).

Reference computation (per batch b):
    keys  = x @ W                       [B,S,D]
    query = tanh(c @ Wc + b)            [B,D]
    eij   = scale * <query, keys_s>     [B,S]
    a     = exp(eij - max) * mask; a /= (sum(a) + 1e-7)
    out   = sum_s a[s] * x[s,:]         [B,D]

Key rewrite: eij[b,s] = <x[b,s,:], q2[b]> with q2[b] = scale * W @ query[b],
removing the [B,S,D]x[D,D] matmul.  v3 additionally:

- Uploads x / W^T / Wc as fp16 from the host (validated: global rel err
  ~2.2e-3 vs the 2e-2 gate).  Halves HBM traffic, halves DVE element work,
  and makes every PE matmul single-pass (fp32 matmuls are 2-pass LOW_HIGH).
- eij on DVE+GpSimd via scalar_tensor_tensor fp16 (accum fp32).
- Pooling in column form on PE: stationary x chunk [128s x 128d] fp16,
  moving a column [128,1] -> psum [128d, 1].  Output lands partition-
  parallel so the softmax-combine tail runs at [128,8] shapes, not [1,1024].
- Per-tile (1024 s) local softmax (local max) + exact flash-style combine
  across the 2 tiles of each batch: removes the "all eij before any pooling"
  serialization; pooling pipelines with the x DMA stream.
- KSHARD=1: W^T/Wc sharded 8-way by e; each core computes partial
  q2 for all 16 batches; ReduceScatter(+) over cores gives each core the
  q2 rows for its 2 batches.  Cuts W+Wc DMA from 4MB to 0.5MB per core.

Sharding: data-parallel over batch: 16 batches / 8 cores = 2 per core.

Per-core x layout (s-major): tile t of batch b is SBUF [128, 8, 1024] fp16
where partition p, free (q, d)  <->  x[b, 1024*t + 8*p + q, d].
"""

import numpy as np
import os

B, S, D = 16, 2048, 1024
NCORES = 8
BPC = B // NCORES  # batches per core
EPS = 1e-7

TS = 2  # x tiles per batch (1024 s each)
QT = 8  # s-rows per partition per tile
SBLK = S // TS  # 1024
KD = D // 128  # 8 chunks of 128 along d/e/c

_CACHE = {}


def _build():
    shard = int(os.environ.get("KSHARD", "1"))
    phase = int(os.environ.get("KPHASE", "5"))
    import concourse.bass as bass
    import concourse.mybir as mybir
    import concourse.tile as tile
    from concourse import bacc
    from concourse.masks import make_identity

    fp32 = mybir.dt.float32
    f16 = mybir.dt.float16
    bf16 = mybir.dt.bfloat16
    i32 = mybir.dt.int32
    AF = mybir.ActivationFunctionType
    OP = mybir.AluOpType
    ts = bass.ts

    nc = bacc.Bacc(None)

    MQ = B if shard else BPC  # batches flowing through the q2 pipeline
    ESH = 128 if shard else D  # e-slice width held by this core
    KE = ESH // 128

    x_d = nc.dram_tensor("x", [BPC, S, D], f16, kind="ExternalInput")
    mask_d = nc.dram_tensor("mask", [BPC, S], i32, kind="ExternalInput")
    c_d = nc.dram_tensor("c", [MQ, D], fp32, kind="ExternalInput")
    # W arrives host-transposed (and e-sliced when shard): w_d[e, d] = W[d, e]
    w_d = nc.dram_tensor("W", [ESH, D], f16, kind="ExternalInput")
    # Wc arrives natural (e-sliced cols when shard): wc_d[c, e]
    wc_d = nc.dram_tensor("Wc", [D, ESH], f16, kind="ExternalInput")
    b_d = nc.dram_tensor("b", [ESH], fp32, kind="ExternalInput")
    scale_d = nc.dram_tensor("scale", [1], fp32, kind="ExternalInput")
    out_d = nc.dram_tensor("out", [BPC, D], fp32, kind="ExternalOutput")

    with tile.TileContext(nc) as tc:
        with (
            tc.tile_pool(name="const", bufs=1) as const,
            tc.tile_pool(name="xp", bufs=BPC * TS) as xp,
            tc.tile_pool(name="wst", bufs=1) as wst,
            tc.tile_pool(name="stat", bufs=2 * TS) as stat,
            tc.tile_pool(name="scr", bufs=2) as scr,
            tc.tile_pool(name="pq", bufs=1, space="PSUM") as pq,
            tc.tile_pool(name="pb", bufs=2, space="PSUM") as pb,
            tc.tile_pool(name="pp", bufs=2, space="PSUM") as pp,
            tc.tile_pool(name="dram", bufs=1, space="DRAM") as dram,
        ):
            # ---------- constants ----------
            identity = const.tile([128, 128], fp32, tag="identity")
            make_identity(nc, identity)
            ones1f = const.tile([1, 128], fp32, tag="ones1f")
            nc.vector.memset(ones1f, 1.0)
            ones1h = const.tile([1, 128], f16, tag="ones1h")
            nc.vector.memset(ones1h, 1.0)
            ones_col = const.tile([128, 1], fp32, tag="ones_col")
            nc.vector.memset(ones_col, 1.0)
            # row-selector: sel2[k, j, m] = (k == j), fp16 (for fp16 rhs)
            sel2 = const.tile([BPC, BPC, 128], f16, tag="sel2")
            nc.gpsimd.memset(sel2, 1.0)
            nc.gpsimd.affine_select(
                out=sel2,
                in_=sel2,
                compare_op=OP.is_equal,
                fill=0.0,
                base=0,
                pattern=[[-1, BPC], [0, 128]],
                channel_multiplier=1,
            )

            scale_sb = const.tile([1, 1], fp32, tag="scale")
            nc.sync.dma_start(out=scale_sb, in_=scale_d[None, :])
            scale128 = const.tile([128, 1], fp32, tag="scale128")
            psc = pb.tile([128, 512], fp32, tag="pb", name="psc")
            nc.tensor.matmul(psc[:, 0:1], ones1f, scale_sb, start=True, stop=True)
            nc.scalar.copy(scale128, psc[:, 0:1])

            # ---- small DMAs needed by the q2 pipeline ----
            c_rows = const.tile([MQ, D], fp32, tag="c_rows")
            nc.sync.dma_start(out=c_rows, in_=c_d[:, :])
            bias_row = const.tile([1, ESH], f16, tag="bias_row")
            nc.gpsimd.dma_start(out=bias_row, in_=b_d[None, :])  # f32->f16 cast

            # Wc then W^T, natural layouts
            wc_sb = wst.tile([128, KD, ESH], f16, tag="wc", name="wc")
            nc.sync.dma_start(
                out=wc_sb,
                in_=wc_d.rearrange("(k p) e -> p k e", p=128),
            )
            wt_sb = wst.tile([128, KE, D], f16, tag="wt", name="wt")
            nc.sync.dma_start(
                out=wt_sb,
                in_=w_d.rearrange("(k p) d -> p k d", p=128),
            )

            # masks (cast int32 -> f32 during DMA); layout matches eij.
            # mask_neg = -1e9 where masked, 0 where kept (for the masked max).
            mask_f = []
            mask_neg = []
            for b2 in range(BPC):
                mf = const.tile([128, TS, QT], fp32, tag=f"mask{b2}")
                nc.gpsimd.dma_start(
                    out=mf,
                    in_=mask_d[b2].rearrange("(t p q) -> p t q", p=128, q=QT),
                )
                mask_f.append(mf)
                mn = const.tile([128, TS, QT], fp32, tag=f"maskn{b2}")
                nc.vector.tensor_scalar(
                    out=mn,
                    in0=mf,
                    scalar1=1.0,
                    scalar2=1e9,
                    op0=OP.subtract,
                    op1=OP.mult,
                )
                mask_neg.append(mn)

            # x tiles (the bulk: 8MB fp16), issued after the weights
            x_tiles = [[None] * TS for _ in range(BPC)]
            for b2 in range(BPC if phase >= 2 else 0):
                for t in range(TS):
                    xt = xp.tile([128, QT, D], f16, tag="xt")
                    nc.sync.dma_start(
                        out=xt,
                        in_=x_d[b2, ts(t, SBLK), :].rearrange(
                            "(p q) d -> p q d", p=128
                        ),
                    )
                    x_tiles[b2][t] = xt

            # ---------- q2 = scale * W @ tanh(c @ Wc + bias) ----------
            # cT[p, kc, m] = c[m, 128*kc + p] via PE transposes
            cT = const.tile([128, KD, MQ], f16, tag="cT")
            for kc in range(KD):
                ptc = pb.tile([128, 512], fp32, tag="pb", name="ptc")
                nc.tensor.transpose(
                    ptc[:, 0:MQ], c_rows[:, ts(kc, 128)], identity[0:MQ, 0:MQ]
                )
                nc.scalar.copy(cT[:, kc, :], ptc[:, 0:MQ])

            # psum_q[m, e] = sum_c c[m, c] * Wc[c, e]  (+ bias)
            psum_q = pq.tile([MQ, ESH], fp32, tag="pq", name="psum_q")
            NES = min(ESH, 512)
            for kc in range(KD):
                for h in range(ESH // NES):
                    nc.tensor.matmul(
                        psum_q[:, ts(h, NES)],
                        cT[:, kc, :],
                        wc_sb[:, kc, ts(h, NES)],
                        start=(kc == 0),
                        stop=False,
                    )
            for h in range(ESH // NES):
                nc.tensor.matmul(
                    psum_q[:, ts(h, NES)],
                    ones1h[0:1, 0:MQ],
                    bias_row[0:1, ts(h, NES)],
                    start=False,
                    stop=True,
                )
            q_pre = const.tile([MQ, ESH], fp32, tag="q_pre")
            nc.scalar.copy(q_pre, psum_q)

            # qT[p, ke, m] = tanh(pre)[m, 128*ke + p]; then partial
            # q2[m, d] = sum_e qT[e, m] * WT[e, d]
            qT = const.tile([128, KE, MQ], f16, tag="qT")
            psum_p = pq.tile([MQ, D], fp32, tag="pq2", name="psum_p")
            for ke in range(KE):
                ptq = pb.tile([128, 512], fp32, tag="pb", name="ptq")
                nc.tensor.transpose(
                    ptq[:, 0:MQ], q_pre[:, ts(ke, 128)], identity[0:MQ, 0:MQ]
                )
                nc.scalar.activation(qT[:, ke, :], ptq[:, 0:MQ], AF.Tanh)
                for h in range(2):
                    nc.tensor.matmul(
                        psum_p[:, ts(h, 512)],
                        qT[:, ke, :],
                        wt_sb[:, ke, ts(h, 512)],
                        start=(ke == 0),
                        stop=(ke == KE - 1),
                    )

            if shard:
                # partial q2 for all 16 batches -> ReduceScatter(+) -> own rows
                q2part = const.tile([MQ, D], fp32, tag="q2part")
                nc.scalar.copy(q2part, psum_p)
                q2p_d = dram.tile([MQ, D], fp32, tag="q2p_d")
                q2g_d = dram.tile([BPC, D], fp32, tag="q2g_d")
                nc.gpsimd.dma_start(out=q2p_d, in_=q2part)
                nc.gpsimd.collective_compute(
                    "ReduceScatter",
                    mybir.AluOpType.add,
                    replica_groups=[list(range(NCORES))],
                    ins=[q2p_d[:, :].opt()],
                    outs=[q2g_d[:, :].opt()],
                )
                q2rs = const.tile([BPC, D], fp32, tag="q2rs")
                nc.gpsimd.dma_start(out=q2rs, in_=q2g_d)
                # fold scale, cast to fp16
                q2row = const.tile([BPC, D], f16, tag="q2row")
                nc.scalar.mul(q2row, q2rs, scale128[0:BPC])
            else:
                q2row = const.tile([BPC, D], f16, tag="q2row")
                nc.scalar.mul(q2row, psum_p, scale128[0:BPC])

            # broadcast q2 rows to 128 partitions (fp16)
            q2b = []
            for b2 in range(BPC):
                qb = const.tile([128, D], f16, tag=f"q2b{b2}", name="qb")
                for h in range(2):
                    pbc = pb.tile([128, 512], fp32, tag="pb", name="pbc")
                    nc.tensor.matmul(
                        pbc,
                        sel2[:, b2, :],
                        q2row[:, ts(h, 512)],
                        start=True,
                        stop=True,
                    )
                    nc.scalar.copy(qb[:, ts(h, 512)], pbc)
                q2b.append(qb)

            if phase == 1:
                for b2 in range(BPC):
                    nc.sync.dma_start(
                        out=out_d[b2 : b2 + 1, 0:512],
                        in_=q2b[b2][0:1, :].bitcast(fp32),
                    )

            # ---------- per-tile: eij, local softmax, pooling ----------
            # All eij dot-products on DVE (the GpSimd ISA rejects
            # fp16 scalar_tensor_tensor with accum).
            def stt_engines(g):
                return [nc.vector] * QT

            negmx = [[None] * TS for _ in range(BPC)]  # [1,1] = -unmasked max
            neghx = [[None] * TS for _ in range(BPC)]  # [1,1] = -survivor max
            s_loc = [[None] * TS for _ in range(BPC)]  # [1,1] = local sum
            po = [[None] * TS for _ in range(BPC)]  # [128, KD] psum pooled

            for g in range(BPC * TS if phase >= 3 else 0):
                b2, t = g // TS, g % TS
                xt = x_tiles[b2][t]
                engines = stt_engines(g)

                # eij[p, q] = <x[s], q2[b]>, s = SBLK*t + QT*p + q
                eij = stat.tile([128, QT], fp32, tag="eij")
                for q in range(QT):
                    eng = engines[q]
                    sc = scr.tile(
                        [128, D],
                        bf16,
                        tag="sttv" if eng is nc.vector else "sttg",
                        bufs=1,
                    )
                    eng.scalar_tensor_tensor(
                        out=sc,
                        in0=xt[:, q, :],
                        scalar=1.0,
                        in1=q2b[b2],
                        op0=OP.mult,
                        op1=OP.mult,
                        accum_out=eij[:, q : q + 1],
                    )

                if phase == 3:
                    if g == 0:
                        nc.sync.dma_start(
                            out=out_d[0:1, 0:QT], in_=eij[0:1, :]
                        )
                    continue

                # local UNMASKED max (for the reference EPS anchoring) and
                # local MASKED max (survivor max; safe fp16 exp anchor).
                m1 = stat.tile([128, 1], fp32, tag="m1")
                nc.vector.reduce_max(m1, eij, axis=mybir.AxisListType.X)
                pmax = pb.tile([128, 512], fp32, tag="pb", name="pmax")
                nc.tensor.transpose(pmax[0:1, 0:128], m1, identity)
                nmx = stat.tile([1, 1], fp32, tag="nmx")
                nc.vector.reduce_max(
                    nmx, pmax[0:1, 0:128], axis=mybir.AxisListType.X, negate=True
                )
                negmx[b2][t] = nmx

                em = stat.tile([128, QT], fp32, tag="em")
                nc.vector.tensor_tensor(em, eij, mask_neg[b2][:, t, :], op=OP.add)
                em1 = stat.tile([128, 1], fp32, tag="em1")
                nc.vector.reduce_max(em1, em, axis=mybir.AxisListType.X)
                pmax2 = pb.tile([128, 512], fp32, tag="pb", name="pmax2")
                nc.tensor.transpose(pmax2[0:1, 0:128], em1, identity)
                nhr = stat.tile([1, 1], fp32, tag="nhr")
                nc.vector.reduce_max(
                    nhr, pmax2[0:1, 0:128], axis=mybir.AxisListType.X, negate=True
                )
                # clamp (negated space): nh = min(nh_raw, n + 80)
                t80 = stat.tile([1, 1], fp32, tag="t80")
                nc.vector.tensor_scalar_add(t80, nmx, 80.0)
                nhx = stat.tile([1, 1], fp32, tag="nhx")
                nc.vector.tensor_tensor(nhx, nhr, t80, op=OP.min)
                neghx[b2][t] = nhx

                pbm = pb.tile([128, 512], fp32, tag="pb", name="pbm")
                nc.tensor.matmul(pbm[:, 0:1], ones1f, nhx, start=True, stop=True)
                negm_b = stat.tile([128, 1], fp32, tag="negm_b")
                nc.scalar.copy(negm_b, pbm[:, 0:1])

                # a = exp(eij - mh_loc) * mask   (fp16 for the PE matmuls)
                a_raw = stat.tile([128, QT], fp32, tag="a_raw")
                nc.scalar.activation(a_raw, eij, AF.Exp, bias=negm_b, scale=1.0)
                a16 = stat.tile([128, QT], f16, tag="a16")
                nc.vector.tensor_tensor(
                    a16, a_raw, mask_f[b2][:, t, :], op=OP.mult
                )

                # local sum S_t (of the fp16-rounded a, matching the pooling)
                s1 = stat.tile([128, 1], fp32, tag="s1")
                nc.vector.reduce_sum(s1, a16, axis=mybir.AxisListType.X)
                pss = pb.tile([128, 512], fp32, tag="pb", name="pss")
                nc.tensor.matmul(pss[0:1, 0:1], s1, ones_col, start=True, stop=True)
                st = stat.tile([1, 1], fp32, tag="st")
                nc.scalar.copy(st, pss[0:1, 0:1])
                s_loc[b2][t] = st

                if phase == 4:
                    if g == 0:
                        nc.sync.dma_start(out=out_d[0:1, 0:QT], in_=a_raw[0:1, :])
                    continue

                # pooling: po[pd, dc] = sum_{q} sum_{ps} x[ps, q, dc*128+pd] * a[ps, q]
                pot = pp.tile([128, KD], fp32, tag="po", name="pot")
                for dc in range(KD):
                    for q in range(QT):
                        nc.tensor.matmul(
                            pot[:, dc : dc + 1],
                            xt[:, q, ts(dc, 128)],
                            a16[:, q : q + 1],
                            start=(q == 0),
                            stop=(q == QT - 1),
                        )
                po[b2][t] = pot

                # ---- per-batch combine after its last tile ----
                if t == TS - 1 and phase >= 5:
                    nmg = stat.tile([1, 1], fp32, tag="nmg")
                    nc.vector.tensor_tensor(
                        nmg, neghx[b2][0], neghx[b2][1], op=OP.min
                    )
                    # f_t = exp(mh_t - mh) = exp(nmg - nh_t)
                    g128 = []
                    fts = []
                    for t2 in range(TS):
                        dlt = stat.tile([1, 1], fp32, tag=f"dlt{t2}")
                        nc.vector.tensor_tensor(
                            dlt, nmg, neghx[b2][t2], op=OP.subtract
                        )
                        ft = stat.tile([1, 1], fp32, tag=f"ft{t2}")
                        nc.scalar.activation(ft, dlt, AF.Exp)
                        fts.append(ft)
                    # epsfac = exp(m - mh) = exp(nmg - ng)  (>= 1)
                    ng = stat.tile([1, 1], fp32, tag="ng")
                    nc.vector.tensor_tensor(
                        ng, negmx[b2][0], negmx[b2][1], op=OP.min
                    )
                    dge = stat.tile([1, 1], fp32, tag="dge")
                    nc.vector.tensor_tensor(dge, nmg, ng, op=OP.subtract)
                    epsv = stat.tile([1, 1], fp32, tag="epsv")
                    nc.scalar.activation(epsv, dge, AF.Exp, scale=1.0)
                    nc.vector.tensor_scalar_mul(epsv, epsv, EPS)
                    # den = f0*S0 + f1*S1 + EPS*epsfac ; rden = 1/den
                    sf0 = stat.tile([1, 1], fp32, tag="sf0")
                    nc.vector.tensor_tensor(sf0, fts[0], s_loc[b2][0], op=OP.mult)
                    sf1 = stat.tile([1, 1], fp32, tag="sf1")
                    nc.vector.tensor_tensor(sf1, fts[1], s_loc[b2][1], op=OP.mult)
                    den = stat.tile([1, 1], fp32, tag="den")
                    nc.vector.tensor_tensor(den, sf0, sf1, op=OP.add)
                    nc.vector.tensor_tensor(den, den, epsv, op=OP.add)
                    rden = stat.tile([1, 1], fp32, tag="rden")
                    nc.vector.reciprocal(rden, den)
                    for t2 in range(TS):
                        gt = stat.tile([1, 1], fp32, tag=f"gt{t2}")
                        nc.vector.tensor_tensor(gt, fts[t2], rden, op=OP.mult)
                        pg = pb.tile([128, 512], fp32, tag="pb", name="pg")
                        nc.tensor.matmul(
                            pg[:, 0:1], ones1f, gt, start=True, stop=True
                        )
                        g1 = stat.tile([128, 1], fp32, tag=f"g128_{t2}")
                        nc.scalar.copy(g1, pg[:, 0:1])
                        g128.append(g1)
                    # res[pd, dc] = g0 * po0 + g1 * po1
                    tmp = stat.tile([128, KD], fp32, tag="cmb_tmp")
                    nc.scalar.mul(tmp, po[b2][0], g128[0])
                    res = stat.tile([128, KD], fp32, tag="cmb_res")
                    nc.vector.scalar_tensor_tensor(
                        out=res,
                        in0=po[b2][1],
                        scalar=g128[1],
                        in1=tmp,
                        op0=OP.mult,
                        op1=OP.add,
                    )
                    # transpose to [KD, 128] rows and DMA out
                    pot_t = pb.tile([128, 512], fp32, tag="pb", name="pot_t")
                    nc.tensor.transpose(
                        pot_t[0:KD, 0:128], res, identity
                    )
                    outrow = stat.tile([KD, 128], fp32, tag="outrow")
                    nc.scalar.copy(outrow, pot_t[0:KD, 0:128])
                    nc.sync.dma_start(
                        out=out_d[b2].rearrange("(dc p) -> dc p", p=128),
                        in_=outrow,
                    )

    nc.compile()
    return nc


def _get_nc():
    if "nc" not in _CACHE:
        _CACHE["nc"] = _build()
    return _CACHE["nc"]


def run(inputs, trace=False):
    from concourse.bass_utils import run_bass_kernel_spmd

    shard = int(os.environ.get("KSHARD", "1"))

    x = np.asarray(inputs["x"], dtype=np.float32).astype(np.float16)
    mask = np.ascontiguousarray(np.asarray(inputs["mask"], dtype=np.int32))
    c = np.ascontiguousarray(np.asarray(inputs["c"], dtype=np.float32))
    WT = np.asarray(inputs["W"], dtype=np.float32).T.astype(np.float16)
    Wc = np.asarray(inputs["Wc"], dtype=np.float32).astype(np.float16)
    bias = np.ascontiguousarray(np.asarray(inputs["b"], dtype=np.float32))
    scale = np.ascontiguousarray(np.asarray(inputs["scale"], dtype=np.float32))

    in_maps = []
    for i in range(NCORES):
        sl = slice(i * BPC, (i + 1) * BPC)
        esl = slice(i * 128, (i + 1) * 128)
        if shard:
            m = {
                "x": np.ascontiguousarray(x[sl]),
                "mask": mask[sl],
                "c": c,
                "W": np.ascontiguousarray(WT[esl, :]),
                "Wc": np.ascontiguousarray(Wc[:, esl]),
                "b": np.ascontiguousarray(bias[esl]),
                "scale": scale,
            }
        else:
            m = {
                "x": np.ascontiguousarray(x[sl]),
                "mask": mask[sl],
                "c": np.ascontiguousarray(c[sl]),
                "W": np.ascontiguousarray(WT),
                "Wc": Wc,
                "b": bias,
                "scale": scale,
            }
        in_maps.append(m)

    nc = _get_nc()
    res = run_bass_kernel_spmd(
        nc, in_maps, core_ids=list(range(NCORES)), trace=trace
    )
    out = np.concatenate([res.results[i]["out"] for i in range(NCORES)], axis=0)
    return out.astype(np.float32), res


def kernel(**inputs):
    out, _ = run(inputs, trace=False)
    return out


# revision 18
# speedup vs baseline: 1.4530x; 1.4530x over previous
"""Trainium2 Bass kernel for ContextAttentionMaskLuong (v3: fp16 streaming).

Reference computation (per batch b):
    keys  = x @ W                       [B,S,D]
    query = tanh(c @ Wc + b)            [B,D]
    eij   = scale * <query, keys_s>     [B,S]
    a     = exp(eij - max) * mask; a /= (sum(a) + 1e-7)
    out   = sum_s a[s] * x[s,:]         [B,D]

Key rewrite: eij[b,s] = <x[b,s,:], q2[b]> with q2[b] = scale * W @ query[b],
removing the [B,S,D]x[D,D] matmul.  v3 additionally:

- Uploads x / W^T / Wc as fp16 from the host (validated: global rel err
  ~2.2e-3 vs the 2e-2 gate).  Halves HBM traffic, halves DVE element work,
  and makes every PE matmul single-pass (fp32 matmuls are 2-pass LOW_HIGH).
- eij on DVE+GpSimd via scalar_tensor_tensor fp16 (accum fp32).
- Pooling in column form on PE: stationary x chunk [128s x 128d] fp16,
  moving a column [128,1] -> psum [128d, 1].  Output lands partition-
  parallel so the softmax-combine tail runs at [128,8] shapes, not [1,1024].
- Per-tile (1024 s) local softmax (local max) + exact flash-style combine
  across the 2 tiles of each batch: removes the "all eij before any pooling"
  serialization; pooling pipelines with the x DMA stream.
- KSHARD=1: W^T/Wc sharded 8-way by e; each core computes partial
  q2 for all 16 batches; ReduceScatter(+) over cores gives each core the
  q2 rows for its 2 batches.  Cuts W+Wc DMA from 4MB to 0.5MB per core.

Sharding: data-parallel over batch: 16 batches / 8 cores = 2 per core.

Per-core x layout (s-major): tile t of batch b is SBUF [128, 8, 1024] fp16
where partition p, free (q, d)  <->  x[b, 1024*t + 8*p + q, d].
"""

import numpy as np
import os

B, S, D = 16, 2048, 1024
NCORES = 8
BPC = B // NCORES  # batches per core
EPS = 1e-7

TS = 2  # x tiles per batch (1024 s each)
QT = 8  # s-rows per partition per tile
SBLK = S // TS  # 1024
KD = D // 128  # 8 chunks of 128 along d/e/c

_CACHE = {}


def _build():
    shard = int(os.environ.get("KSHARD", "0"))
    phase = int(os.environ.get("KPHASE", "5"))
    keij = os.environ.get("KEIJ", "ttr")  # stt | ttr | hyb
    kxsplit = int(os.environ.get("KXSPLIT", "1"))
    kwsplit = int(os.environ.get("KWSPLIT", "1"))
    import concourse.bass as bass
    import concourse.mybir as mybir
    import concourse.tile as tile
    from concourse import bacc
    from concourse.masks import make_identity

    fp32 = mybir.dt.float32
    f16 = mybir.dt.float16
    bf16 = mybir.dt.bfloat16
    i32 = mybir.dt.int32
    AF = mybir.ActivationFunctionType
    OP = mybir.AluOpType
    ts = bass.ts

    nc = bacc.Bacc(None)

    MQ = B if shard else BPC  # batches flowing through the q2 pipeline
    ESH = 128 if shard else D  # e-slice width held by this core
    KE = ESH // 128

    x_d = nc.dram_tensor("x", [BPC, S, D], f16, kind="ExternalInput")
    mask_d = nc.dram_tensor("mask", [BPC, S], i32, kind="ExternalInput")
    c_d = nc.dram_tensor("c", [MQ, D], fp32, kind="ExternalInput")
    # W arrives host-transposed (and e-sliced when shard): w_d[e, d] = W[d, e]
    w_d = nc.dram_tensor("W", [ESH, D], f16, kind="ExternalInput")
    # Wc arrives natural (e-sliced cols when shard): wc_d[c, e]
    wc_d = nc.dram_tensor("Wc", [D, ESH], f16, kind="ExternalInput")
    b_d = nc.dram_tensor("b", [ESH], fp32, kind="ExternalInput")
    scale_d = nc.dram_tensor("scale", [1], fp32, kind="ExternalInput")
    out_d = nc.dram_tensor("out", [BPC, D], fp32, kind="ExternalOutput")

    with tile.TileContext(nc) as tc:
        with (
            tc.tile_pool(name="const", bufs=1) as const,
            tc.tile_pool(name="xp", bufs=BPC * TS) as xp,
            tc.tile_pool(name="wst", bufs=1) as wst,
            tc.tile_pool(name="stat", bufs=2 * TS) as stat,
            tc.tile_pool(name="scr", bufs=2) as scr,
            tc.tile_pool(name="pq", bufs=1, space="PSUM") as pq,
            tc.tile_pool(name="pb", bufs=2, space="PSUM") as pb,
            tc.tile_pool(name="pp", bufs=2, space="PSUM") as pp,
            tc.tile_pool(name="dram", bufs=1, space="DRAM") as dram,
        ):
            # ---------- constants ----------
            identity = const.tile([128, 128], fp32, tag="identity")
            make_identity(nc, identity)
            ones1f = const.tile([1, 128], fp32, tag="ones1f")
            nc.vector.memset(ones1f, 1.0)
            ones1h = const.tile([1, 128], f16, tag="ones1h")
            nc.vector.memset(ones1h, 1.0)
            ones_col = const.tile([128, 1], fp32, tag="ones_col")
            nc.vector.memset(ones_col, 1.0)
            # row-selector: sel2[k, j, m] = (k == j), fp16 (for fp16 rhs)
            sel2 = const.tile([BPC, BPC, 128], f16, tag="sel2")
            nc.gpsimd.memset(sel2, 1.0)
            nc.gpsimd.affine_select(
                out=sel2,
                in_=sel2,
                compare_op=OP.is_equal,
                fill=0.0,
                base=0,
                pattern=[[-1, BPC], [0, 128]],
                channel_multiplier=1,
            )

            scale_sb = const.tile([1, 1], fp32, tag="scale")
            nc.sync.dma_start(out=scale_sb, in_=scale_d[None, :])
            scale128 = const.tile([128, 1], fp32, tag="scale128")
            psc = pb.tile([128, 512], fp32, tag="pb", name="psc")
            nc.tensor.matmul(psc[:, 0:1], ones1f, scale_sb, start=True, stop=True)
            nc.scalar.copy(scale128, psc[:, 0:1])

            # ---- small DMAs needed by the q2 pipeline ----
            c_rows = const.tile([MQ, D], fp32, tag="c_rows")
            nc.sync.dma_start(out=c_rows, in_=c_d[:, :])
            bias_row = const.tile([1, ESH], f16, tag="bias_row")
            nc.gpsimd.dma_start(out=bias_row, in_=b_d[None, :])  # f32->f16 cast

            # Wc then W^T, natural layouts, halved DMAs so the q2 matmuls
            # can start on the first half while the second streams in
            wc_sb = wst.tile([128, KD, ESH], f16, tag="wc", name="wc")
            wc_src = wc_d.rearrange("(k p) e -> p k e", p=128)
            HK = max(KD // 2, 1) if kwsplit else KD
            for hh in range(KD // HK):
                nc.sync.dma_start(
                    out=wc_sb[:, ts(hh, HK), :], in_=wc_src[:, ts(hh, HK), :]
                )
            wt_sb = wst.tile([128, KE, D], f16, tag="wt", name="wt")
            wt_src = w_d.rearrange("(k p) d -> p k d", p=128)
            HE = max(KE // 2, 1) if kwsplit else KE
            for hh in range(KE // HE):
                nc.sync.dma_start(
                    out=wt_sb[:, ts(hh, HE), :], in_=wt_src[:, ts(hh, HE), :]
                )

            # masks (cast int32 -> f32 during DMA); layout matches eij.
            # mask_neg = -1e9 where masked, 0 where kept (for the masked max).
            mask_f = []
            mask_neg = []
            for b2 in range(BPC):
                mf = const.tile([128, TS, QT], fp32, tag=f"mask{b2}")
                nc.gpsimd.dma_start(
                    out=mf,
                    in_=mask_d[b2].rearrange("(t p q) -> p t q", p=128, q=QT),
                )
                mask_f.append(mf)
                mn = const.tile([128, TS, QT], fp32, tag=f"maskn{b2}")
                nc.vector.tensor_scalar(
                    out=mn,
                    in0=mf,
                    scalar1=1.0,
                    scalar2=1e9,
                    op0=OP.subtract,
                    op1=OP.mult,
                )
                mask_neg.append(mn)

            # x tiles (the bulk: 8MB fp16), issued after the weights.
            # Each tile lands as two 1MB half-DMAs so eij slices for the
            # first half start while the second half streams.
            x_tiles = [[None] * TS for _ in range(BPC)]
            for b2 in range(BPC if phase >= 2 else 0):
                for t in range(TS):
                    xt = xp.tile([128, QT, D], f16, tag="xt")
                    src = x_d[b2, ts(t, SBLK), :].rearrange(
                        "(p q) d -> p q d", p=128
                    )
                    if kxsplit:
                        hq = QT // 2
                        nc.sync.dma_start(out=xt[:, 0:hq, :], in_=src[:, 0:hq, :])
                        nc.sync.dma_start(
                            out=xt[:, hq:QT, :], in_=src[:, hq:QT, :]
                        )
                    else:
                        nc.sync.dma_start(out=xt, in_=src)
                    x_tiles[b2][t] = xt

            # ---------- q2 = scale * W @ tanh(c @ Wc + bias) ----------
            # cT[p, kc, m] = c[m, 128*kc + p] via PE transposes
            cT = const.tile([128, KD, MQ], f16, tag="cT")
            for kc in range(KD):
                ptc = pb.tile([128, 512], fp32, tag="pb", name="ptc")
                nc.tensor.transpose(
                    ptc[:, 0:MQ], c_rows[:, ts(kc, 128)], identity[0:MQ, 0:MQ]
                )
                nc.scalar.copy(cT[:, kc, :], ptc[:, 0:MQ])

            # psum_q[m, e] = sum_c c[m, c] * Wc[c, e]  (+ bias)
            psum_q = pq.tile([MQ, ESH], fp32, tag="pq", name="psum_q")
            NES = min(ESH, 512)
            for kc in range(KD):
                for h in range(ESH // NES):
                    nc.tensor.matmul(
                        psum_q[:, ts(h, NES)],
                        cT[:, kc, :],
                        wc_sb[:, kc, ts(h, NES)],
                        start=(kc == 0),
                        stop=False,
                    )
            for h in range(ESH // NES):
                nc.tensor.matmul(
                    psum_q[:, ts(h, NES)],
                    ones1h[0:1, 0:MQ],
                    bias_row[0:1, ts(h, NES)],
                    start=False,
                    stop=True,
                )
            q_pre = const.tile([MQ, ESH], fp32, tag="q_pre")
            nc.scalar.copy(q_pre, psum_q)

            # qT[p, ke, m] = tanh(pre)[m, 128*ke + p]; then partial
            # q2[m, d] = sum_e qT[e, m] * WT[e, d]
            qT = const.tile([128, KE, MQ], f16, tag="qT")
            psum_p = pq.tile([MQ, D], fp32, tag="pq2", name="psum_p")
            for ke in range(KE):
                ptq = pb.tile([128, 512], fp32, tag="pb", name="ptq")
                nc.tensor.transpose(
                    ptq[:, 0:MQ], q_pre[:, ts(ke, 128)], identity[0:MQ, 0:MQ]
                )
                nc.scalar.activation(qT[:, ke, :], ptq[:, 0:MQ], AF.Tanh)
                for h in range(2):
                    nc.tensor.matmul(
                        psum_p[:, ts(h, 512)],
                        qT[:, ke, :],
                        wt_sb[:, ke, ts(h, 512)],
                        start=(ke == 0),
                        stop=(ke == KE - 1),
                    )

            if shard:
                # partial q2 for all 16 batches -> ReduceScatter(+) -> own rows
                q2part = const.tile([MQ, D], fp32, tag="q2part")
                nc.scalar.copy(q2part, psum_p)
                q2p_d = dram.tile([MQ, D], fp32, tag="q2p_d")
                q2g_d = dram.tile([BPC, D], fp32, tag="q2g_d")
                nc.gpsimd.dma_start(out=q2p_d, in_=q2part)
                nc.gpsimd.collective_compute(
                    "ReduceScatter",
                    mybir.AluOpType.add,
                    replica_groups=[list(range(NCORES))],
                    ins=[q2p_d[:, :].opt()],
                    outs=[q2g_d[:, :].opt()],
                )
                q2rs = const.tile([BPC, D], fp32, tag="q2rs")
                nc.gpsimd.dma_start(out=q2rs, in_=q2g_d)
                # fold scale, cast to fp16
                q2row = const.tile([BPC, D], f16, tag="q2row")
                nc.scalar.mul(q2row, q2rs, scale128[0:BPC])
            else:
                q2row = const.tile([BPC, D], f16, tag="q2row")
                nc.scalar.mul(q2row, psum_p, scale128[0:BPC])

            # broadcast q2 rows to 128 partitions (fp16)
            q2b = []
            for b2 in range(BPC):
                qb = const.tile([128, D], f16, tag=f"q2b{b2}", name="qb")
                for h in range(2):
                    pbc = pb.tile([128, 512], fp32, tag="pb", name="pbc")
                    nc.tensor.matmul(
                        pbc,
                        sel2[:, b2, :],
                        q2row[:, ts(h, 512)],
                        start=True,
                        stop=True,
                    )
                    nc.scalar.copy(qb[:, ts(h, 512)], pbc)
                q2b.append(qb)

            if phase == 1:
                for b2 in range(BPC):
                    nc.sync.dma_start(
                        out=out_d[b2 : b2 + 1, 0:512],
                        in_=q2b[b2][0:1, :].bitcast(fp32),
                    )

            # ---------- per-tile: eij, local softmax, pooling ----------
            # eij dot-product implementations (see KEIJ):
            #  stt: DVE scalar_tensor_tensor w/ accum (1x mode, ~1.22us/slice)
            #  ttr: DVE tensor_tensor_reduce (maybe a faster uop tier)
            #  hyb: 5 slices DVE-stt + 3 slices GpSimd-product + ACT-accum
            def eij_slice(eng_kind, xt, b2, eij, q):
                if eng_kind == "ttr":
                    sc = scr.tile([128, D], bf16, tag="sttv", bufs=1)
                    nc.vector.tensor_tensor_reduce(
                        out=sc,
                        in0=xt[:, q, :],
                        in1=q2b[b2],
                        scale=1.0,
                        scalar=0.0,
                        op0=OP.mult,
                        op1=OP.add,
                        accum_out=eij[:, q : q + 1],
                    )
                elif eng_kind == "gact":
                    pr = scr.tile([128, D], f16, tag="gprod", bufs=2)
                    nc.gpsimd.tensor_tensor(
                        pr, xt[:, q, :], q2b[b2], op=OP.mult
                    )
                    sc = scr.tile([128, D], f16, tag="ascr", bufs=1)
                    nc.scalar.activation(
                        sc, pr, AF.Copy, accum_out=eij[:, q : q + 1]
                    )
                else:  # stt on DVE
                    sc = scr.tile([128, D], bf16, tag="sttv", bufs=1)
                    nc.vector.scalar_tensor_tensor(
                        out=sc,
                        in0=xt[:, q, :],
                        scalar=1.0,
                        in1=q2b[b2],
                        op0=OP.mult,
                        op1=OP.mult,
                        accum_out=eij[:, q : q + 1],
                    )

            def stt_kinds(g):
                if keij == "ttr":
                    return ["ttr"] * QT
                if keij == "hyb":
                    return ["stt"] * 5 + ["gact"] * 3
                return ["stt"] * QT

            negmx = [[None] * TS for _ in range(BPC)]  # [1,1] = -unmasked max
            neghx = [[None] * TS for _ in range(BPC)]  # [1,1] = -survivor max
            s_loc = [[None] * TS for _ in range(BPC)]  # [1,1] = local sum
            po = [[None] * TS for _ in range(BPC)]  # [128, KD] psum pooled

            for g in range(BPC * TS if phase >= 3 else 0):
                b2, t = g // TS, g % TS
                xt = x_tiles[b2][t]
                kinds = stt_kinds(g)

                # eij[p, q] = <x[s], q2[b]>, s = SBLK*t + QT*p + q
                eij = stat.tile([128, QT], fp32, tag="eij")
                for q in range(QT):
                    eij_slice(kinds[q], xt, b2, eij, q)

                if phase == 3:
                    if g == 0:
                        nc.sync.dma_start(
                            out=out_d[0:1, 0:QT], in_=eij[0:1, :]
                        )
                    continue

                # local UNMASKED max (for the reference EPS anchoring) and
                # local MASKED max (survivor max; safe fp16 exp anchor).
                m1 = stat.tile([128, 1], fp32, tag="m1")
                nc.vector.reduce_max(m1, eij, axis=mybir.AxisListType.X)
                pmax = pb.tile([128, 512], fp32, tag="pb", name="pmax")
                nc.tensor.transpose(pmax[0:1, 0:128], m1, identity)
                nmx = stat.tile([1, 1], fp32, tag="nmx")
                nc.vector.reduce_max(
                    nmx, pmax[0:1, 0:128], axis=mybir.AxisListType.X, negate=True
                )
                negmx[b2][t] = nmx

                em = stat.tile([128, QT], fp32, tag="em")
                nc.vector.tensor_tensor(em, eij, mask_neg[b2][:, t, :], op=OP.add)
                em1 = stat.tile([128, 1], fp32, tag="em1")
                nc.vector.reduce_max(em1, em, axis=mybir.AxisListType.X)
                pmax2 = pb.tile([128, 512], fp32, tag="pb", name="pmax2")
                nc.tensor.transpose(pmax2[0:1, 0:128], em1, identity)
                nhr = stat.tile([1, 1], fp32, tag="nhr")
                nc.vector.reduce_max(
                    nhr, pmax2[0:1, 0:128], axis=mybir.AxisListType.X, negate=True
                )
                # clamp (negated space): nh = min(nh_raw, n + 80)
                t80 = stat.tile([1, 1], fp32, tag="t80")
                nc.vector.tensor_scalar_add(t80, nmx, 80.0)
                nhx = stat.tile([1, 1], fp32, tag="nhx")
                nc.vector.tensor_tensor(nhx, nhr, t80, op=OP.min)
                neghx[b2][t] = nhx

                pbm = pb.tile([128, 512], fp32, tag="pb", name="pbm")
                nc.tensor.matmul(pbm[:, 0:1], ones1f, nhx, start=True, stop=True)
                negm_b = stat.tile([128, 1], fp32, tag="negm_b")
                nc.scalar.copy(negm_b, pbm[:, 0:1])

                # a = exp(eij - mh_loc) * mask   (fp16 for the PE matmuls)
                a_raw = stat.tile([128, QT], fp32, tag="a_raw")
                nc.scalar.activation(a_raw, eij, AF.Exp, bias=negm_b, scale=1.0)
                a16 = stat.tile([128, QT], f16, tag="a16")
                nc.vector.tensor_tensor(
                    a16, a_raw, mask_f[b2][:, t, :], op=OP.mult
                )

                # local sum S_t (of the fp16-rounded a, matching the pooling)
                s1 = stat.tile([128, 1], fp32, tag="s1")
                nc.vector.reduce_sum(s1, a16, axis=mybir.AxisListType.X)
                pss = pb.tile([128, 512], fp32, tag="pb", name="pss")
                nc.tensor.matmul(pss[0:1, 0:1], s1, ones_col, start=True, stop=True)
                st = stat.tile([1, 1], fp32, tag="st")
                nc.scalar.copy(st, pss[0:1, 0:1])
                s_loc[b2][t] = st

                if phase == 4:
                    if g == 0:
                        nc.sync.dma_start(out=out_d[0:1, 0:QT], in_=a_raw[0:1, :])
                    continue

                # pooling: po[pd, dc] = sum_{q} sum_{ps} x[ps, q, dc*128+pd] * a[ps, q]
                pot = pp.tile([128, KD], fp32, tag="po", name="pot")
                for dc in range(KD):
                    for q in range(QT):
                        nc.tensor.matmul(
                            pot[:, dc : dc + 1],
                            xt[:, q, ts(dc, 128)],
                            a16[:, q : q + 1],
                            start=(q == 0),
                            stop=(q == QT - 1),
                        )
                po[b2][t] = pot

                # ---- per-batch combine after its last tile ----
                if t == TS - 1 and phase >= 5:
                    nmg = stat.tile([1, 1], fp32, tag="nmg")
                    nc.vector.tensor_tensor(
                        nmg, neghx[b2][0], neghx[b2][1], op=OP.min
                    )
                    # f_t = exp(mh_t - mh) = exp(nmg - nh_t)
                    g128 = []
                    fts = []
                    for t2 in range(TS):
                        dlt = stat.tile([1, 1], fp32, tag=f"dlt{t2}")
                        nc.vector.tensor_tensor(
                            dlt, nmg, neghx[b2][t2], op=OP.subtract
                        )
                        ft = stat.tile([1, 1], fp32, tag=f"ft{t2}")
                        nc.scalar.activation(ft, dlt, AF.Exp)
                        fts.append(ft)
                    # epsfac = exp(m - mh) = exp(nmg - ng)  (>= 1)
                    ng = stat.tile([1, 1], fp32, tag="ng")
                    nc.vector.tensor_tensor(
                        ng, negmx[b2][0], negmx[b2][1], op=OP.min
                    )
                    dge = stat.tile([1, 1], fp32, tag="dge")
                    nc.vector.tensor_tensor(dge, nmg, ng, op=OP.subtract)
                    epsv = stat.tile([1, 1], fp32, tag="epsv")
                    nc.scalar.activation(epsv, dge, AF.Exp, scale=1.0)
                    nc.vector.tensor_scalar_mul(epsv, epsv, EPS)
                    # den = f0*S0 + f1*S1 + EPS*epsfac ; rden = 1/den
                    sf0 = stat.tile([1, 1], fp32, tag="sf0")
                    nc.vector.tensor_tensor(sf0, fts[0], s_loc[b2][0], op=OP.mult)
                    sf1 = stat.tile([1, 1], fp32, tag="sf1")
                    nc.vector.tensor_tensor(sf1, fts[1], s_loc[b2][1], op=OP.mult)
                    den = stat.tile([1, 1], fp32, tag="den")
                    nc.vector.tensor_tensor(den, sf0, sf1, op=OP.add)
                    nc.vector.tensor_tensor(den, den, epsv, op=OP.add)
                    rden = stat.tile([1, 1], fp32, tag="rden")
                    nc.vector.reciprocal(rden, den)
                    for t2 in range(TS):
                        gt = stat.tile([1, 1], fp32, tag=f"gt{t2}")
                        nc.vector.tensor_tensor(gt, fts[t2], rden, op=OP.mult)
                        pg = pb.tile([128, 512], fp32, tag="pb", name="pg")
                        nc.tensor.matmul(
                            pg[:, 0:1], ones1f, gt, start=True, stop=True
                        )
                        g1 = stat.tile([128, 1], fp32, tag=f"g128_{t2}")
                        nc.scalar.copy(g1, pg[:, 0:1])
                        g128.append(g1)
                    # res[pd, dc] = g0 * po0 + g1 * po1
                    tmp = stat.tile([128, KD], fp32, tag="cmb_tmp")
                    nc.scalar.mul(tmp, po[b2][0], g128[0])
                    res = stat.tile([128, KD], fp32, tag="cmb_res")
                    nc.vector.scalar_tensor_tensor(
                        out=res,
                        in0=po[b2][1],
                        scalar=g128[1],
                        in1=tmp,
                        op0=OP.mult,
                        op1=OP.add,
                    )
                    # transpose to [KD, 128] rows and DMA out
                    pot_t = pb.tile([128, 512], fp32, tag="pb", name="pot_t")
                    nc.tensor.transpose(
                        pot_t[0:KD, 0:128], res, identity
                    )
                    outrow = stat.tile([KD, 128], fp32, tag="outrow")
                    nc.scalar.copy(outrow, pot_t[0:KD, 0:128])
                    nc.sync.dma_start(
                        out=out_d[b2].rearrange("(dc p) -> dc p", p=128),
                        in_=outrow,
                    )

    nc.compile()
    return nc


def _get_nc():
    if "nc" not in _CACHE:
        _CACHE["nc"] = _build()
    return _CACHE["nc"]


def run(inputs, trace=False):
    from concourse.bass_utils import run_bass_kernel_spmd

    shard = int(os.environ.get("KSHARD", "0"))

    x = np.asarray(inputs["x"], dtype=np.float32).astype(np.float16)
    mask = np.ascontiguousarray(np.asarray(inputs["mask"], dtype=np.int32))
    c = np.ascontiguousarray(np.asarray(inputs["c"], dtype=np.float32))
    WT = np.asarray(inputs["W"], dtype=np.float32).T.astype(np.float16)
    Wc = np.asarray(inputs["Wc"], dtype=np.float32).astype(np.float16)
    bias = np.ascontiguousarray(np.asarray(inputs["b"], dtype=np.float32))
    scale = np.ascontiguousarray(np.asarray(inputs["scale"], dtype=np.float32))

    in_maps = []
    for i in range(NCORES):
        sl = slice(i * BPC, (i + 1) * BPC)
        esl = slice(i * 128, (i + 1) * 128)
        if shard:
            m = {
                "x": np.ascontiguousarray(x[sl]),
                "mask": mask[sl],
                "c": c,
                "W": np.ascontiguousarray(WT[esl, :]),
                "Wc": np.ascontiguousarray(Wc[:, esl]),
                "b": np.ascontiguousarray(bias[esl]),
                "scale": scale,
            }
        else:
            m = {
                "x": np.ascontiguousarray(x[sl]),
                "mask": mask[sl],
                "c": np.ascontiguousarray(c[sl]),
                "W": np.ascontiguousarray(WT),
                "Wc": Wc,
                "b": bias,
                "scale": scale,
            }
        in_maps.append(m)

    nc = _get_nc()
    res = run_bass_kernel_spmd(
        nc, in_maps, core_ids=list(range(NCORES)), trace=trace
    )
    out = np.concatenate([res.results[i]["out"] for i in range(NCORES)], axis=0)
    return out.astype(np.float32), res


def kernel(**inputs):
    out, _ = run(inputs, trace=False)
    return out


# revision 42
# speedup vs baseline: 1.5121x; 1.0407x over previous
"""Trainium2 Bass kernel for ContextAttentionMaskLuong (v3: fp16 streaming).

Reference computation (per batch b):
    keys  = x @ W                       [B,S,D]
    query = tanh(c @ Wc + b)            [B,D]
    eij   = scale * <query, keys_s>     [B,S]
    a     = exp(eij - max) * mask; a /= (sum(a) + 1e-7)
    out   = sum_s a[s] * x[s,:]         [B,D]

Key rewrite: eij[b,s] = <x[b,s,:], q2[b]> with q2[b] = scale * W @ query[b],
removing the [B,S,D]x[D,D] matmul.  v3 additionally:

- Uploads x / W^T / Wc as fp16 from the host (validated: global rel err
  ~2.2e-3 vs the 2e-2 gate).  Halves HBM traffic, halves DVE element work,
  and makes every PE matmul single-pass (fp32 matmuls are 2-pass LOW_HIGH).
- eij on DVE+GpSimd via scalar_tensor_tensor fp16 (accum fp32).
- Pooling in column form on PE: stationary x chunk [128s x 128d] fp16,
  moving a column [128,1] -> psum [128d, 1].  Output lands partition-
  parallel so the softmax-combine tail runs at [128,8] shapes, not [1,1024].
- Per-tile (1024 s) local softmax (local max) + exact flash-style combine
  across the 2 tiles of each batch: removes the "all eij before any pooling"
  serialization; pooling pipelines with the x DMA stream.
- KSHARD=1: W^T/Wc sharded 8-way by e; each core computes partial
  q2 for all 16 batches; ReduceScatter(+) over cores gives each core the
  q2 rows for its 2 batches.  Cuts W+Wc DMA from 4MB to 0.5MB per core.

Sharding: data-parallel over batch: 16 batches / 8 cores = 2 per core.

Per-core x layout (s-major): tile t of batch b is SBUF [128, 8, 1024] fp16
where partition p, free (q, d)  <->  x[b, 1024*t + 8*p + q, d].
"""

import numpy as np
import os

B, S, D = 16, 2048, 1024
NCORES = 8
BPC = B // NCORES  # batches per core
EPS = 1e-7

TS = 2  # x tiles per batch (1024 s each)
QT = 8  # s-rows per partition per tile
SBLK = S // TS  # 1024
KD = D // 128  # 8 chunks of 128 along d/e/c

_CACHE = {}


def _build():
    shard = int(os.environ.get("KSHARD", "0"))
    phase = int(os.environ.get("KPHASE", "5"))
    keij = os.environ.get("KEIJ", "stt")  # stt | ttr | hyb
    kxsplit = int(os.environ.get("KXSPLIT", "1"))
    kwsplit = int(os.environ.get("KWSPLIT", "1"))
    kgate = int(os.environ.get("KGATE", "1"))
    kxq = os.environ.get("KXQ", "gp")  # gp | sync
    import concourse.bass as bass
    import concourse.mybir as mybir
    import concourse.tile as tile
    from concourse import bacc
    from concourse.masks import make_identity

    fp32 = mybir.dt.float32
    f16 = mybir.dt.float16
    bf16 = mybir.dt.bfloat16
    i32 = mybir.dt.int32
    AF = mybir.ActivationFunctionType
    OP = mybir.AluOpType
    ts = bass.ts

    nc = bacc.Bacc(None)

    MQ = B if shard else BPC  # batches flowing through the q2 pipeline
    ESH = 128 if shard else D  # e-slice width held by this core
    KE = ESH // 128

    x_d = nc.dram_tensor("x", [BPC, S, D], f16, kind="ExternalInput")
    mask_d = nc.dram_tensor("mask", [BPC, S], i32, kind="ExternalInput")
    c_d = nc.dram_tensor("c", [MQ, D], fp32, kind="ExternalInput")
    # W arrives host-transposed (and e-sliced when shard): w_d[e, d] = W[d, e]
    w_d = nc.dram_tensor("W", [ESH, D], f16, kind="ExternalInput")
    # Wc arrives natural (e-sliced cols when shard): wc_d[c, e]
    wc_d = nc.dram_tensor("Wc", [D, ESH], f16, kind="ExternalInput")
    b_d = nc.dram_tensor("b", [ESH], fp32, kind="ExternalInput")
    scale_d = nc.dram_tensor("scale", [1], fp32, kind="ExternalInput")
    out_d = nc.dram_tensor("out", [BPC, D], fp32, kind="ExternalOutput")

    with tile.TileContext(nc) as tc:
        with (
            tc.tile_pool(name="const", bufs=1) as const,
            tc.tile_pool(name="xp", bufs=BPC * TS) as xp,
            tc.tile_pool(name="wst", bufs=1) as wst,
            tc.tile_pool(name="stat", bufs=2 * TS) as stat,
            tc.tile_pool(name="scr", bufs=2) as scr,
            tc.tile_pool(name="pq", bufs=1, space="PSUM") as pq,
            tc.tile_pool(name="pb", bufs=2, space="PSUM") as pb,
            tc.tile_pool(name="pp", bufs=2, space="PSUM") as pp,
            tc.tile_pool(name="dram", bufs=1, space="DRAM") as dram,
        ):
            # ---------- constants ----------
            identity = const.tile([128, 128], fp32, tag="identity")
            make_identity(nc, identity)
            ones1f = const.tile([1, 128], fp32, tag="ones1f")
            nc.vector.memset(ones1f, 1.0)
            ones1h = const.tile([1, 128], f16, tag="ones1h")
            nc.vector.memset(ones1h, 1.0)
            ones_col = const.tile([128, 1], fp32, tag="ones_col")
            nc.vector.memset(ones_col, 1.0)
            # row-selector: sel2[k, j, m] = (k == j), fp16 (for fp16 rhs)
            sel2 = const.tile([BPC, BPC, 128], f16, tag="sel2")
            nc.gpsimd.memset(sel2, 1.0)
            nc.gpsimd.affine_select(
                out=sel2,
                in_=sel2,
                compare_op=OP.is_equal,
                fill=0.0,
                base=0,
                pattern=[[-1, BPC], [0, 128]],
                channel_multiplier=1,
            )

            scale_sb = const.tile([1, 1], fp32, tag="scale")
            nc.sync.dma_start(out=scale_sb, in_=scale_d[None, :])
            scale128 = const.tile([128, 1], fp32, tag="scale128")
            psc = pb.tile([128, 512], fp32, tag="pb", name="psc")
            nc.tensor.matmul(psc[:, 0:1], ones1f, scale_sb, start=True, stop=True)
            nc.scalar.copy(scale128, psc[:, 0:1])

            # ---- small DMAs needed by the q2 pipeline ----
            c_rows = const.tile([MQ, D], fp32, tag="c_rows")
            nc.sync.dma_start(out=c_rows, in_=c_d[:, :])
            bias_row = const.tile([1, ESH], f16, tag="bias_row")
            nc.gpsimd.dma_start(out=bias_row, in_=b_d[None, :])  # f32->f16 cast

            # Wc then W^T, natural layouts, halved DMAs so the q2 matmuls
            # can start on the first half while the second streams in
            wc_sb = wst.tile([128, KD, ESH], f16, tag="wc", name="wc")
            wc_src = wc_d.rearrange("(k p) e -> p k e", p=128)
            HK = max(KD // 2, 1) if kwsplit else KD
            for hh in range(KD // HK):
                nc.sync.dma_start(
                    out=wc_sb[:, ts(hh, HK), :], in_=wc_src[:, ts(hh, HK), :]
                )
            wt_sb = wst.tile([128, KE, D], f16, tag="wt", name="wt")
            wt_src = w_d.rearrange("(k p) d -> p k d", p=128)
            HE = max(KE // 2, 1) if kwsplit else KE
            for hh in range(KE // HE):
                nc.sync.dma_start(
                    out=wt_sb[:, ts(hh, HE), :], in_=wt_src[:, ts(hh, HE), :]
                )

            # masks (cast int32 -> f32 during DMA); layout matches eij.
            # mask_neg = -1e9 where masked, 0 where kept (for the masked max).
            mask_f = []
            mask_neg = []
            for b2 in range(BPC):
                mf = const.tile([128, TS, QT], fp32, tag=f"mask{b2}")
                nc.gpsimd.dma_start(
                    out=mf,
                    in_=mask_d[b2].rearrange("(t p q) -> p t q", p=128, q=QT),
                )
                mask_f.append(mf)
                mn = const.tile([128, TS, QT], fp32, tag=f"maskn{b2}")
                nc.vector.tensor_scalar(
                    out=mn,
                    in0=mf,
                    scalar1=1.0,
                    scalar2=1e9,
                    op0=OP.subtract,
                    op1=OP.mult,
                )
                mask_neg.append(mn)

            # x tiles (the bulk: 8MB fp16), issued after the weights.
            # Each tile lands as two 1MB half-DMAs so eij slices for the
            # first half start while the second half streams.
            #
            # HW DMA transfers round-robin across queued descriptors, so
            # merely issuing W first does NOT prioritize it — the x stream
            # steals ~2/3 of HBM bandwidth and the latency-critical W path
            # lands ~3x late.  Force the first x DMA (and the sync FIFO
            # behind it) to wait for the W data with artificial 1-element
            # reads of the last-queued W chunks.
            # The x stream goes through the GpSimd (SWDGE) queue, gated on
            # the W data by 1-element reads, so the W path gets the full
            # HBM bandwidth first.  Everything stays on one engine queue
            # (no cross-queue blocking -> no deadlock).
            x_tiles = [[None] * TS for _ in range(BPC)]
            first_x = [True]

            xeng = nc.gpsimd if kxq == "gp" else nc.sync

            def w_gate(xt):
                if first_x[0] and phase >= 2 and kgate:
                    first_x[0] = False
                    nc.gpsimd.tensor_scalar_add(
                        xt[0:1, 0:1, 0:1], wc_sb[0:1, KD - 1, 0:1], 0.0
                    )
                    nc.gpsimd.tensor_scalar_add(
                        xt[0:1, 0:1, 1:2], wt_sb[0:1, KE - 1, 0:1], 0.0
                    )

            for b2 in range(BPC if phase >= 2 else 0):
                for t in range(TS):
                    xt = xp.tile([128, QT, D], f16, tag="xt")
                    w_gate(xt)
                    src = x_d[b2, ts(t, SBLK), :].rearrange(
                        "(p q) d -> p q d", p=128
                    )
                    if kxsplit:
                        hq = QT // 2
                        xeng.dma_start(out=xt[:, 0:hq, :], in_=src[:, 0:hq, :])
                        xeng.dma_start(out=xt[:, hq:QT, :], in_=src[:, hq:QT, :])
                    else:
                        xeng.dma_start(out=xt, in_=src)
                    x_tiles[b2][t] = xt

            # ---------- q2 = scale * W @ tanh(c @ Wc + bias) ----------
            # cT[p, kc, m] = c[m, 128*kc + p] via PE transposes
            cT = const.tile([128, KD, MQ], f16, tag="cT")
            for kc in range(KD):
                ptc = pb.tile([128, 512], fp32, tag="pb", name="ptc")
                nc.tensor.transpose(
                    ptc[:, 0:MQ], c_rows[:, ts(kc, 128)], identity[0:MQ, 0:MQ]
                )
                nc.scalar.copy(cT[:, kc, :], ptc[:, 0:MQ])

            # psum_q[m, e] = sum_c c[m, c] * Wc[c, e]  (+ bias)
            psum_q = pq.tile([MQ, ESH], fp32, tag="pq", name="psum_q")
            NES = min(ESH, 512)
            for kc in range(KD):
                for h in range(ESH // NES):
                    nc.tensor.matmul(
                        psum_q[:, ts(h, NES)],
                        cT[:, kc, :],
                        wc_sb[:, kc, ts(h, NES)],
                        start=(kc == 0),
                        stop=False,
                    )
            for h in range(ESH // NES):
                nc.tensor.matmul(
                    psum_q[:, ts(h, NES)],
                    ones1h[0:1, 0:MQ],
                    bias_row[0:1, ts(h, NES)],
                    start=False,
                    stop=True,
                )
            q_pre = const.tile([MQ, ESH], fp32, tag="q_pre")
            nc.scalar.copy(q_pre, psum_q)

            # qT[p, ke, m] = tanh(pre)[m, 128*ke + p]; then partial
            # q2[m, d] = sum_e qT[e, m] * WT[e, d]
            qT = const.tile([128, KE, MQ], f16, tag="qT")
            psum_p = pq.tile([MQ, D], fp32, tag="pq2", name="psum_p")
            for ke in range(KE):
                ptq = pb.tile([128, 512], fp32, tag="pb", name="ptq")
                nc.tensor.transpose(
                    ptq[:, 0:MQ], q_pre[:, ts(ke, 128)], identity[0:MQ, 0:MQ]
                )
                nc.scalar.activation(qT[:, ke, :], ptq[:, 0:MQ], AF.Tanh)
                for h in range(2):
                    nc.tensor.matmul(
                        psum_p[:, ts(h, 512)],
                        qT[:, ke, :],
                        wt_sb[:, ke, ts(h, 512)],
                        start=(ke == 0),
                        stop=(ke == KE - 1),
                    )

            if shard:
                # partial q2 for all 16 batches -> ReduceScatter(+) -> own rows
                q2part = const.tile([MQ, D], fp32, tag="q2part")
                nc.scalar.copy(q2part, psum_p)
                q2p_d = dram.tile([MQ, D], fp32, tag="q2p_d")
                q2g_d = dram.tile([BPC, D], fp32, tag="q2g_d")
                nc.gpsimd.dma_start(out=q2p_d, in_=q2part)
                nc.gpsimd.collective_compute(
                    "ReduceScatter",
                    mybir.AluOpType.add,
                    replica_groups=[list(range(NCORES))],
                    ins=[q2p_d[:, :].opt()],
                    outs=[q2g_d[:, :].opt()],
                )
                q2rs = const.tile([BPC, D], fp32, tag="q2rs")
                nc.gpsimd.dma_start(out=q2rs, in_=q2g_d)
                # fold scale, cast to fp16
                q2row = const.tile([BPC, D], f16, tag="q2row")
                nc.scalar.mul(q2row, q2rs, scale128[0:BPC])
            else:
                q2row = const.tile([BPC, D], f16, tag="q2row")
                nc.scalar.mul(q2row, psum_p, scale128[0:BPC])

            # broadcast q2 rows to 128 partitions (fp16); split the psum->
            # sbuf copies across ACT and DVE so batch 0's q2b (which gates
            # the first eij) is ready as early as possible
            q2b = []
            for b2 in range(BPC):
                qb = const.tile([128, D], f16, tag=f"q2b{b2}", name="qb")
                for h in range(2):
                    pbc = pb.tile([128, 512], fp32, tag="pb", name="pbc")
                    nc.tensor.matmul(
                        pbc,
                        sel2[:, b2, :],
                        q2row[:, ts(h, 512)],
                        start=True,
                        stop=True,
                    )
                    nc.scalar.copy(qb[:, ts(h, 512)], pbc)
                q2b.append(qb)

            if phase == 1:
                for b2 in range(BPC):
                    nc.sync.dma_start(
                        out=out_d[b2 : b2 + 1, 0:512],
                        in_=q2b[b2][0:1, :].bitcast(fp32),
                    )

            # ---------- per-tile: eij, local softmax, pooling ----------
            # eij dot-product implementations (see KEIJ):
            #  stt: DVE scalar_tensor_tensor w/ accum (1x mode, ~1.22us/slice)
            #  ttr: DVE tensor_tensor_reduce (maybe a faster uop tier)
            #  hyb: 5 slices DVE-stt + 3 slices GpSimd-product + ACT-accum
            def eij_slice(eng_kind, xt, b2, eij, q):
                if eng_kind == "ttr":
                    sc = scr.tile([128, D], bf16, tag="sttv", bufs=1)
                    nc.vector.tensor_tensor_reduce(
                        out=sc,
                        in0=xt[:, q, :],
                        in1=q2b[b2],
                        scale=1.0,
                        scalar=0.0,
                        op0=OP.mult,
                        op1=OP.add,
                        accum_out=eij[:, q : q + 1],
                    )
                elif eng_kind == "gact":
                    pr = scr.tile([128, D], f16, tag="gprod", bufs=2)
                    nc.gpsimd.tensor_tensor(
                        pr, xt[:, q, :], q2b[b2], op=OP.mult
                    )
                    sc = scr.tile([128, D], f16, tag="ascr", bufs=1)
                    nc.scalar.activation(
                        sc, pr, AF.Copy, accum_out=eij[:, q : q + 1]
                    )
                else:  # stt on DVE
                    sc = scr.tile([128, D], bf16, tag="sttv", bufs=1)
                    nc.vector.scalar_tensor_tensor(
                        out=sc,
                        in0=xt[:, q, :],
                        scalar=1.0,
                        in1=q2b[b2],
                        op0=OP.mult,
                        op1=OP.mult,
                        accum_out=eij[:, q : q + 1],
                    )

            def stt_kinds(g):
                if keij == "ttr":
                    return ["ttr"] * QT
                if keij == "hyb":
                    return ["stt"] * 5 + ["gact"] * 3
                return ["stt"] * QT

            negmx = [[None] * TS for _ in range(BPC)]  # [1,1] = -unmasked max
            neghx = [[None] * TS for _ in range(BPC)]  # [1,1] = -survivor max
            s_loc = [[None] * TS for _ in range(BPC)]  # [1,1] = local sum
            po = [[None] * TS for _ in range(BPC)]  # [128, KD] psum pooled

            for g in range(BPC * TS if phase >= 3 else 0):
                b2, t = g // TS, g % TS
                xt = x_tiles[b2][t]
                kinds = stt_kinds(g)

                # eij[p, q] = <x[s], q2[b]>, s = SBLK*t + QT*p + q
                eij = stat.tile([128, QT], fp32, tag="eij")
                for q in range(QT):
                    eij_slice(kinds[q], xt, b2, eij, q)

                if phase == 3:
                    if g == 0:
                        nc.sync.dma_start(
                            out=out_d[0:1, 0:QT], in_=eij[0:1, :]
                        )
                    continue

                # em = eij - 1e9*(1-mask): masked entries sink to -inf-ish,
                # so exp(em - survivor_max) is exactly 0 for them and <= 1
                # for survivors -> fp16-safe, no clamp, no post-mask.
                em = stat.tile([128, QT], fp32, tag="em")
                nc.vector.tensor_tensor(em, eij, mask_neg[b2][:, t, :], op=OP.add)
                em1 = stat.tile([128, 1], fp32, tag="em1")
                nc.vector.reduce_max(em1, em, axis=mybir.AxisListType.X)
                pmax2 = pb.tile([128, 512], fp32, tag="pb", name="pmax2")
                nc.tensor.transpose(pmax2[0:1, 0:128], em1, identity)
                nhx = stat.tile([1, 1], fp32, tag="nhx")
                nc.vector.reduce_max(
                    nhx, pmax2[0:1, 0:128], axis=mybir.AxisListType.X, negate=True
                )
                neghx[b2][t] = nhx

                # local UNMASKED max (only for the reference EPS anchoring;
                # off the critical path, consumed at combine time)
                m1 = stat.tile([128, 1], fp32, tag="m1")
                nc.vector.reduce_max(m1, eij, axis=mybir.AxisListType.X)
                pmax = pb.tile([128, 512], fp32, tag="pb", name="pmax")
                nc.tensor.transpose(pmax[0:1, 0:128], m1, identity)
                nmx = stat.tile([1, 1], fp32, tag="nmx")
                nc.vector.reduce_max(
                    nmx, pmax[0:1, 0:128], axis=mybir.AxisListType.X, negate=True
                )
                negmx[b2][t] = nmx

                pbm = pb.tile([128, 512], fp32, tag="pb", name="pbm")
                nc.tensor.matmul(pbm[:, 0:1], ones1f, nhx, start=True, stop=True)
                negm_b = stat.tile([128, 1], fp32, tag="negm_b")
                nc.scalar.copy(negm_b, pbm[:, 0:1])

                # a = exp(em - mh_loc), directly in fp16 for the PE matmuls
                a16 = stat.tile([128, QT], f16, tag="a16")
                nc.scalar.activation(a16, em, AF.Exp, bias=negm_b, scale=1.0)

                # local sum S_t (of the fp16-rounded a, matching the pooling)
                s1 = stat.tile([128, 1], fp32, tag="s1")
                nc.vector.reduce_sum(s1, a16, axis=mybir.AxisListType.X)
                pss = pb.tile([128, 512], fp32, tag="pb", name="pss")
                nc.tensor.matmul(pss[0:1, 0:1], s1, ones_col, start=True, stop=True)
                st = stat.tile([1, 1], fp32, tag="st")
                nc.scalar.copy(st, pss[0:1, 0:1])
                s_loc[b2][t] = st

                if phase == 4:
                    if g == 0:
                        nc.sync.dma_start(out=out_d[0:1, 0:4], in_=em[0:1, 0:4])
                    continue

                # pooling: po[pd, dc] = sum_{q} sum_{ps} x[ps, q, dc*128+pd] * a[ps, q]
                pot = pp.tile([128, KD], fp32, tag="po", name="pot")
                for dc in range(KD):
                    for q in range(QT):
                        nc.tensor.matmul(
                            pot[:, dc : dc + 1],
                            xt[:, q, ts(dc, 128)],
                            a16[:, q : q + 1],
                            start=(q == 0),
                            stop=(q == QT - 1),
                        )
                po[b2][t] = pot

                # ---- per-batch combine after its last tile ----
                if t == TS - 1 and phase >= 5:
                    nmg = stat.tile([1, 1], fp32, tag="nmg")
                    nc.vector.tensor_tensor(
                        nmg, neghx[b2][0], neghx[b2][1], op=OP.min
                    )
                    # f_t = exp(mh_t - mh) = exp(nmg - nh_t)
                    g128 = []
                    fts = []
                    for t2 in range(TS):
                        dlt = stat.tile([1, 1], fp32, tag=f"dlt{t2}")
                        nc.vector.tensor_tensor(
                            dlt, nmg, neghx[b2][t2], op=OP.subtract
                        )
                        ft = stat.tile([1, 1], fp32, tag=f"ft{t2}")
                        nc.scalar.activation(ft, dlt, AF.Exp)
                        fts.append(ft)
                    # epsfac = exp(m - mh) = exp(nmg - ng)  (>= 1)
                    ng = stat.tile([1, 1], fp32, tag="ng")
                    nc.vector.tensor_tensor(
                        ng, negmx[b2][0], negmx[b2][1], op=OP.min
                    )
                    dge = stat.tile([1, 1], fp32, tag="dge")
                    nc.vector.tensor_tensor(dge, nmg, ng, op=OP.subtract)
                    epsv = stat.tile([1, 1], fp32, tag="epsv")
                    nc.scalar.activation(epsv, dge, AF.Exp, scale=1.0)
                    nc.vector.tensor_scalar_mul(epsv, epsv, EPS)
                    # den = f0*S0 + f1*S1 + EPS*epsfac ; rden = 1/den
                    sf0 = stat.tile([1, 1], fp32, tag="sf0")
                    nc.vector.tensor_tensor(sf0, fts[0], s_loc[b2][0], op=OP.mult)
                    sf1 = stat.tile([1, 1], fp32, tag="sf1")
                    nc.vector.tensor_tensor(sf1, fts[1], s_loc[b2][1], op=OP.mult)
                    den = stat.tile([1, 1], fp32, tag="den")
                    nc.vector.tensor_tensor(den, sf0, sf1, op=OP.add)
                    nc.vector.tensor_tensor(den, den, epsv, op=OP.add)
                    rden = stat.tile([1, 1], fp32, tag="rden")
                    nc.vector.reciprocal(rden, den)
                    for t2 in range(TS):
                        gt = stat.tile([1, 1], fp32, tag=f"gt{t2}")
                        nc.vector.tensor_tensor(gt, fts[t2], rden, op=OP.mult)
                        pg = pb.tile([128, 512], fp32, tag="pb", name="pg")
                        nc.tensor.matmul(
                            pg[:, 0:1], ones1f, gt, start=True, stop=True
                        )
                        g1 = stat.tile([128, 1], fp32, tag=f"g128_{t2}")
                        nc.scalar.copy(g1, pg[:, 0:1])
                        g128.append(g1)
                    # res[pd, dc] = g0 * po0 + g1 * po1
                    tmp = stat.tile([128, KD], fp32, tag="cmb_tmp")
                    nc.scalar.mul(tmp, po[b2][0], g128[0])
                    res = stat.tile([128, KD], fp32, tag="cmb_res")
                    nc.vector.scalar_tensor_tensor(
                        out=res,
                        in0=po[b2][1],
                        scalar=g128[1],
                        in1=tmp,
                        op0=OP.mult,
                        op1=OP.add,
                    )
                    # transpose to [KD, 128] rows and DMA out
                    pot_t = pb.tile([128, 512], fp32, tag="pb", name="pot_t")
                    nc.tensor.transpose(
                        pot_t[0:KD, 0:128], res, identity
                    )
                    outrow = stat.tile([KD, 128], fp32, tag="outrow")
                    nc.scalar.copy(outrow, pot_t[0:KD, 0:128])
                    nc.sync.dma_start(
                        out=out_d[b2].rearrange("(dc p) -> dc p", p=128),
                        in_=outrow,
                    )

    nc.compile()
    return nc


def _get_nc():
    if "nc" not in _CACHE:
        _CACHE["nc"] = _build()
    return _CACHE["nc"]


def run(inputs, trace=False):
    from concourse.bass_utils import run_bass_kernel_spmd

    shard = int(os.environ.get("KSHARD", "0"))

    x = np.asarray(inputs["x"], dtype=np.float32).astype(np.float16)
    mask = np.ascontiguousarray(np.asarray(inputs["mask"], dtype=np.int32))
    c = np.ascontiguousarray(np.asarray(inputs["c"], dtype=np.float32))
    WT = np.asarray(inputs["W"], dtype=np.float32).T.astype(np.float16)
    Wc = np.asarray(inputs["Wc"], dtype=np.float32).astype(np.float16)
    bias = np.ascontiguousarray(np.asarray(inputs["b"], dtype=np.float32))
    scale = np.ascontiguousarray(np.asarray(inputs["scale"], dtype=np.float32))

    in_maps = []
    for i in range(NCORES):
        sl = slice(i * BPC, (i + 1) * BPC)
        esl = slice(i * 128, (i + 1) * 128)
        if shard:
            m = {
                "x": np.ascontiguousarray(x[sl]),
                "mask": mask[sl],
                "c": c,
                "W": np.ascontiguousarray(WT[esl, :]),
                "Wc": np.ascontiguousarray(Wc[:, esl]),
                "b": np.ascontiguousarray(bias[esl]),
                "scale": scale,
            }
        else:
            m = {
                "x": np.ascontiguousarray(x[sl]),
                "mask": mask[sl],
                "c": np.ascontiguousarray(c[sl]),
                "W": np.ascontiguousarray(WT),
                "Wc": Wc,
                "b": bias,
                "scale": scale,
            }
        in_maps.append(m)

    nc = _get_nc()
    res = run_bass_kernel_spmd(
        nc, in_maps, core_ids=list(range(NCORES)), trace=trace
    )
    out = np.concatenate([res.results[i]["out"] for i in range(NCORES)], axis=0)
    return out.astype(np.float32), res


def kernel(**inputs):
    out, _ = run(inputs, trace=False)
    return out
